# revision 9
# baseline (speedup 1.0000x reference)
"""Fused ArcFace + batch-hard-triplet combined loss on 8 TRN2 NeuronCores.

Sharding: ArcFace class dimension (50000) split 6250/core (padded to 6272);
embeddings replicated; triplet 2048x2048 distance matrix row-sharded 256/core.

v11: device is a pure DMA -> matmul -> exp machine. All normalization,
transposition and scaling happen in host prep (W rows normalized + transposed
+ bf16; emb pre-normalized and pre-scaled by ARC_SCALE so the exp stream is
exp(pm - 64) with a constant bias). The label logit, phi margin,
sum-of-cosines and triplet finals are computed in the host combine from tiny
per-core partials, so the Scalar engine runs a single Exp table load plus the
exp stream and nothing else. 2048-wide PSUM pieces (3 per B-tile) minimize
ACT instruction count; the 49th class tile is batched as one 16x128 exp.
Outputs use p-major DRAM layouts (128 contiguous descriptors, no 4B-element
descriptor storms); host decodes. Inputs ship as bf16/fp16 to halve DMA.
"""
import math
import os
import sys

import numpy as np
import ml_dtypes

for _p in ("/opt/trn_rl_repo", os.path.expanduser("~/.axon_site/_ro/trn_rl_repo")):
    if _p not in sys.path and os.path.isdir(_p):
        sys.path.insert(0, _p)

B, D, C = 2048, 128, 50000
NCORES = 8
CSH = C // NCORES            # 6250
CPAD = 6272                  # 49 tiles of 128
NBT = 16                     # B tiles of 128 rows
RB = B // NCORES             # 256 triplet rows per core
PW = 2048                    # stream piece width (4 PSUM banks)
NPIECE = 3                   # 3 * 2048 = 6144, + 128-wide tile 48

ARC_MARGIN, ARC_SCALE = 0.5, 64.0
COS_M, SIN_M = math.cos(ARC_MARGIN), math.sin(ARC_MARGIN)
TH = math.cos(math.pi - ARC_MARGIN)
MM = math.sin(math.pi - ARC_MARGIN) * ARC_MARGIN
LABEL_SMOOTH = 0.1
TRIPLET_MARGIN = 0.3
W_ARC, W_TRI = 1.0, 0.5
BIG = 1e9

_CACHE = {}


def _build_nc():
    from contextlib import ExitStack

    from concourse import bacc, mybir, tile

    f32 = mybir.dt.float32
    f16 = mybir.dt.float16
    bf16 = mybir.dt.bfloat16
    A = mybir.AluOpType
    AF = mybir.ActivationFunctionType
    X = mybir.AxisListType.X

    nc = bacc.Bacc("TRN2", target_bir_lowering=False, debug=False,
                   num_devices=NCORES)

    # inputs (host-prepped, see _host_prep)
    wt_d = nc.dram_tensor("wt", [128, CPAD], bf16, kind="ExternalInput").ap()
    ea_d = nc.dram_tensor("ea", [128, B], bf16, kind="ExternalInput").ap()
    er_d = nc.dram_tensor("er", [128, B], bf16, kind="ExternalInput").ap()
    ebt_d = nc.dram_tensor("ebt", [128, RB], bf16, kind="ExternalInput").ap()
    sqc_d = nc.dram_tensor("sqc", [B], f16, kind="ExternalInput").ap()
    labc_d = nc.dram_tensor("labc", [B], f16, kind="ExternalInput").ap()
    ssb_d = nc.dram_tensor("ssb", [128, 2], f32, kind="ExternalInput").ap()
    labb_d = nc.dram_tensor("labb", [128, 2], f32, kind="ExternalInput").ap()
    # outputs, p-major (decoded on host)
    o_se = nc.dram_tensor("sumexp", [128, 4, NBT], f32,
                          kind="ExternalOutput").ap()
    o_tri = nc.dram_tensor("tri", [128, 6], f32, kind="ExternalOutput").ap()

    with tile.TileContext(nc) as tc, ExitStack() as ctx:
        sing = ctx.enter_context(tc.tile_pool(name="sing", bufs=1))
        tmp = ctx.enter_context(tc.tile_pool(name="tmp", bufs=2))
        ps_main = ctx.enter_context(tc.tile_pool(name="psm", bufs=2, space="PSUM"))

        cb_m64 = sing.tile([128, 1], f32)
        nc.vector.memset(cb_m64, -float(ARC_SCALE))

        # ---- input tiles; t48 cols + arc lhs + stream piece 0 land first
        wT = sing.tile([128, CPAD], bf16)
        eA = sing.tile([128, B], bf16)
        nc.sync.dma_start(out=eA, in_=ea_d)
        nc.sync.dma_start(out=wT[:, 6144:CPAD], in_=wt_d[:, 6144:CPAD])
        nc.sync.dma_start(out=wT[:, 0:PW], in_=wt_d[:, 0:PW])
        eR = sing.tile([128, B], bf16)
        nc.sync.dma_start(out=eR, in_=er_d)
        eBT = sing.tile([128, RB], bf16)
        nc.sync.dma_start(out=eBT, in_=ebt_d)
        SQB = sing.tile([128, B], f16)
        nc.sync.dma_start(out=SQB, in_=sqc_d.partition_broadcast(128))
        LABB = sing.tile([128, B], f16)
        nc.sync.dma_start(out=LABB, in_=labc_d.partition_broadcast(128))
        ssB = sing.tile([128, 2], f32)
        nc.sync.dma_start(out=ssB, in_=ssb_d)
        labB = sing.tile([128, 2], f32)
        nc.sync.dma_start(out=labB, in_=labb_d)
        nc.sync.dma_start(out=wT[:, PW:2 * PW], in_=wt_d[:, PW:2 * PW])
        nc.sync.dma_start(out=wT[:, 2 * PW:6144], in_=wt_d[:, 2 * PW:6144])

        acc = sing.tile([128, 4, NBT], f32)
        hp4 = sing.tile([128, 2, 4], f32)
        hn4 = sing.tile([128, 2, 4], f32)
        sm4 = sing.tile([128, 2, 4], f32)
        hfin = sing.tile([128, 6], f32)

        # ---- triplet chunk: rows block k (128), cols chunk j (512).
        # The matmul borrows a 512-col region of the NEXT stream slot (pmn),
        # so the PSUM pool parity is untouched; the region is handed back via
        # a WAR dep once d2p has read it.
        def tri_chunk(k, j, pmn, reg):
            col = slice(512 * j, 512 * j + 512)
            pr = pmn[:, 512 * reg:512 * reg + 512]
            nc.tensor.matmul(pr, eBT[:, 128 * k:128 * k + 128], eR[:, col],
                             start=True, stop=True)
            d2p = tmp.tile([128, 512], bf16, tag="d2p")
            nc.vector.scalar_tensor_tensor(out=d2p, in0=pr, scalar=-2.0,
                                           in1=SQB[:, col], op0=A.mult,
                                           op1=A.add)
            nc.vector.tensor_scalar(out=d2p, in0=d2p, scalar1=ssB[:, k:k + 1],
                                    scalar2=None, op0=A.add)
            sm = tmp.tile([128, 512], bf16, tag="sm")
            nc.vector.tensor_scalar(out=sm, in0=LABB[:, col],
                                    scalar1=labB[:, k:k + 1], scalar2=None,
                                    op0=A.is_equal)
            nc.vector.tensor_reduce(out=sm4[:, k, j:j + 1], in_=sm, axis=X,
                                    op=A.add)
            scrb = tmp.tile([128, 512], bf16, tag="scrb")
            nc.vector.tensor_tensor(out=scrb, in0=d2p, in1=sm, op=A.mult)
            nc.vector.tensor_reduce(out=hp4[:, k, j:j + 1], in_=scrb, axis=X,
                                    op=A.max)
            dnb = tmp.tile([128, 512], bf16, tag="dnb")
            nc.vector.scalar_tensor_tensor(out=dnb, in0=sm, scalar=BIG,
                                           in1=d2p, op0=A.mult, op1=A.add)
            nc.vector.tensor_reduce(out=hn4[:, k, j:j + 1], in_=dnb, axis=X,
                                    op=A.min)

        # two tri chunks after these (pi, bt) stream steps
        tri_pairs = {
            (1, 3): ((0, 0), (0, 1)), (1, 7): ((0, 2), (0, 3)),
            (1, 11): ((1, 0), (1, 1)), (1, 15): ((1, 2), (1, 3)),
        }

        def tri_finals():
            for k in range(2):
                nc.vector.tensor_reduce(out=hfin[:, 0 + k:1 + k],
                                        in_=hp4[:, k, :], axis=X, op=A.max)
                nc.vector.tensor_reduce(out=hfin[:, 2 + k:3 + k],
                                        in_=hn4[:, k, :], axis=X, op=A.min)
                nc.vector.tensor_reduce(out=hfin[:, 4 + k:5 + k],
                                        in_=sm4[:, k, :], axis=X, op=A.add)
            nc.sync.dma_start(out=o_tri, in_=hfin)

        # ---- class tile 48 (cols 6144:6272) first: all 16 bt in one exp,
        # while the stream piece 0 DMA is still in flight
        pt = ps_main.tile([128, PW], f32, tag="pm")
        for bt in range(NBT):
            nc.tensor.matmul(pt[:, 128 * bt:128 * bt + 128],
                             eA[:, 128 * bt:128 * bt + 128],
                             wT[:, 6144:6272], start=True, stop=True)
        nc.scalar.activation(out=pt, in_=pt, func=AF.Exp, bias=cb_m64)
        nc.vector.tensor_reduce(out=acc[:, 3, :],
                                in_=pt.rearrange("a (t d) -> a t d", d=128),
                                axis=X, op=A.add)
        nc.sync.dma_start(out=o_se[:, 3, :], in_=acc[:, 3, :])

        # ---- main stream: matmul -> exp, nothing else on ACT
        pending = None
        for pi in range(NPIECE):
            for bt in range(NBT):
                lhs = eA[:, 128 * bt:128 * bt + 128]
                if pending is not None:
                    pm = pending
                    order = (2, 3, 0, 1)
                    pending = None
                else:
                    pm = ps_main.tile([128, PW], f32, tag="pm", name="pm")
                    order = range(4)
                for m_ in order:
                    nc.tensor.matmul(pm[:, 512 * m_:512 * m_ + 512], lhs,
                                     wT[:, PW * pi + 512 * m_:
                                        PW * pi + 512 * m_ + 512],
                                     start=True, stop=True)
                nc.scalar.activation(out=pm, in_=pm, func=AF.Exp,
                                     bias=cb_m64,
                                     accum_out=acc[:, pi, bt:bt + 1])
                pair = tri_pairs.get((pi, bt))
                if pair is not None:
                    pmn = ps_main.tile([128, PW], f32, tag="pm")
                    tri_chunk(*pair[0], pmn, 0)
                    tri_chunk(*pair[1], pmn, 1)
                    pending = pmn
                    if (pi, bt) == (1, 15):
                        tri_finals()
            nc.sync.dma_start(out=o_se[:, pi, :], in_=acc[:, pi, :])

    nc.compile()
    return nc


def _get_nc():
    if "nc" not in _CACHE:
        _CACHE["nc"] = _build_nc()
    return _CACHE["nc"]


def _host_prep(embeddings, arcface_weight_mat, labels):
    emb = np.ascontiguousarray(embeddings, dtype=np.float32)
    W = np.ascontiguousarray(arcface_weight_mat, dtype=np.float32)
    lab = np.asarray(labels).astype(np.int64)

    en = np.sqrt((emb * emb).sum(axis=1)) + 1e-12          # [B]
    wn = W / (np.sqrt((W * W).sum(axis=1, keepdims=True)) + 1e-12)  # [C, D]

    ea = (ARC_SCALE * emb / en[:, None]).T                 # [D, B]
    ea_bf = np.ascontiguousarray(ea, dtype=ml_dtypes.bfloat16)
    er_bf = np.ascontiguousarray(emb.T, dtype=ml_dtypes.bfloat16)
    sq = (emb * emb).sum(axis=1).astype(np.float32)        # ||e||^2
    sqc = sq.astype(np.float16)
    labc = lab.astype(np.float16)

    in_maps = []
    for c in range(NCORES):
        wt = np.zeros((128, CPAD), dtype=ml_dtypes.bfloat16)
        wt[:, :CSH] = wn[c * CSH:(c + 1) * CSH].T.astype(ml_dtypes.bfloat16)
        rows = slice(c * RB, (c + 1) * RB)
        ebt = np.ascontiguousarray(emb[rows].T, dtype=ml_dtypes.bfloat16)
        ssb = np.ascontiguousarray(sq[rows].reshape(2, 128).T,
                                   dtype=np.float32)
        labb = np.ascontiguousarray(lab[rows].astype(np.float32)
                                    .reshape(2, 128).T)
        in_maps.append({
            "wt": wt, "ea": ea_bf, "er": er_bf, "ebt": ebt,
            "sqc": sqc, "labc": labc, "ssb": ssb, "labb": labb,
        })
    return in_maps, (emb, en, wn, lab, sq)


def _combine(results, host):
    emb, en, wn, lab, sq = host
    B_ = emb.shape[0]
    s = ARC_SCALE

    # label cosine + phi margin (exact, f64)
    cl = (emb * wn[lab]).sum(axis=1).astype(np.float64) / en
    sine = np.sqrt(np.clip(1.0 - cl * cl, 0.0, 1.0))
    phi0 = cl * COS_M - sine * SIN_M
    phi = np.where(cl > TH, phi0, cl - MM)

    # sum of cosines per row (for label smoothing)
    svec = wn.sum(axis=0)                                  # [D]
    csum = (emb @ svec).astype(np.float64) / en + (phi - cl)

    S = np.zeros(B_, np.float64)
    for r in results:
        # o_se[p, s, t]: partial s of row 128*t + p
        S += r["sumexp"].astype(np.float64).sum(axis=1).T.ravel()
    S += np.exp(s * phi - s) - np.exp(s * cl - s)
    lse = s + np.log(S)
    nll = lse - s * phi
    smooth = lse - s * csum / C
    arc = np.mean((1.0 - LABEL_SMOOTH) * nll + LABEL_SMOOTH * smooth)

    # triplet finals; o_tri[p, 0:2]=hp2(k), [p, 2:4]=hn2(k), [p, 4:6]=count(k)
    tri_sum = 0.0
    val_sum = 0.0
    for r in results:
        t = r["tri"].astype(np.float64)
        hp2 = t[:, 0:2].T.ravel()
        hn2 = t[:, 2:4].T.ravel()
        nv = t[:, 4:6].T.ravel()
        dp = np.sqrt(np.maximum(hp2, 0.0) + 1e-16)
        dn = np.sqrt(np.maximum(hn2, 0.0) + 1e-16)
        loss = np.maximum(dp - dn + TRIPLET_MARGIN, 0.0)
        valid = (nv - 1.0) > 0.0
        tri_sum += (loss * valid).sum()
        val_sum += valid.sum()
    tri = tri_sum / max(val_sum, 1.0) if val_sum > 0 else 0.0

    return np.array(W_ARC * arc + W_TRI * tri, dtype=np.float32)


def run_kernel(embeddings, arcface_weight_mat, labels, trace=False):
    """Returns (loss, BassKernelResults)."""
    from concourse.bass_utils import run_bass_kernel_spmd

    nc = _get_nc()
    in_maps, host = _host_prep(embeddings, arcface_weight_mat, labels)
    res = run_bass_kernel_spmd(nc, in_maps, list(range(NCORES)), trace=trace)
    return _combine(res.results, host), res


def kernel(embeddings, arcface_weight_mat, labels):
    out, _ = run_kernel(embeddings, arcface_weight_mat, labels)
    return out
